# revision 29
# baseline (speedup 1.0000x reference)
"""Trainium2 Bass kernel for nn_MultiHeadAttention_Linear_11312943857747.

Math (B=4, S=4096, DM=1024, H=16, HD=64):
    q = softmax(x @ Wq.T + bq) over head_dim
    k = softmax(x @ Wk.T + bk) over seq_len
    v = x @ Wv.T + bv
    gmap[b,h] = k[b,h].T @ v[b,h]            (HD x HD per head)
    o[b,h]    = q[b,h] @ gmap[b,h]
    out = LayerNorm(x + o) * gamma + beta

Sharding: 8 cores = 4 batches x 2 sequence-halves. Each core projects its
2048 rows; the per-head kT@v reduction over the full sequence is completed
with a packed 266KB AllReduce between the two cores sharing a batch
(replica groups [[0,1],[2,3],[4,5],[6,7]]).

Both softmaxes fold into matmuls; exp() needs no max-subtraction
(|q|,|k| <~ 4, softmax is shift-invariant).  Projections run fp8e4 with
DoubleRow (256-deep contraction, weights host-scaled by SW=256; the 1/SW
unscale folds into the exp() activation scale and the v-eviction).

Schedule:
  sweep 1: k/v projections per block; per-head G matmuls (col-tiled,
           DoubleRow over block pairs) accumulate directly in PSUM across
           all 8 block-pairs - no vector adds.  x residual blocks prefetch
           on the idle DMA queue throughout.
  G is copied packed [128,8,65] (only diagonal 64x64 blocks + colsum) and
  AllReduced while the whole q phase runs.
  q phase (per 512-row chunk): q projection -> exp; per-head softmax
           denominators via a mask-stationary matmul into [16,512] PSUM;
           ACT reciprocal; PE broadcast back to [128,512]; DVE scales
           exp(q) by 1/den -> eq_n (fp8).  All softmax normalization of q
           is finished here, hidden under the projection matmul stream.
  sweep 2: per block: 4 pair-stacked DoubleRow o-matmuls (N=256) of
           eq_n @ g2 (g2 = G/colsum, block-diagonal, fp8); ACT evicts
           po -> o (bf16); DVE adds the residual with accum_out giving
           sum(y) for free; GpSimd computes sum(y^2); LN closes with an
           ACT rsqrt + one DVE tensor_scalar (4x mode) normalize.
"""

import sys

sys.path.insert(0, "/opt/trn_rl_repo")

import numpy as np
from contextlib import ExitStack

import concourse.bass as bass
import concourse.mybir as mybir
import concourse.tile as tile
from concourse.bass_utils import run_bass_kernel_spmd

F32 = mybir.dt.float32
BF16 = mybir.dt.bfloat16
F8 = mybir.dt.float8e4
DR = mybir.MatmulPerfMode.DoubleRow
AF = mybir.ActivationFunctionType
ALU = mybir.AluOpType

B, S, DM, H, HD = 4, 4096, 1024, 16, 64
EPS = 1e-5
NCORES = 8
R = S // 2          # rows per core
P = 128             # partitions
NBLK = R // P       # 16 sequence blocks of 128 rows
NKT = DM // P       # 8 k-tiles over the contraction dim
NK2 = NKT // 2      # 4 double-k-tiles (256 contraction per DoubleRow pass)
NPAIR = DM // P     # 8 head-pairs (2 heads of 64 = 128 channels)
CHUNK = 512         # moving-operand width for the big projections
NCHUNK = R // CHUNK # 4
BPC = CHUNK // P    # 4 blocks per chunk
SW = 256.0          # host-side weight scale for fp8 range
ISW = 1.0 / SW
INV_N = 1.0 / DM


def _fix_multiwaits(nc):
    """This walrus build encodes at most one sync wait per instruction;
    split any multi-wait instruction into preceding same-engine NoOps."""
    for fn in nc.m.functions:
        for bb in fn.blocks:
            new_insts = []
            changed = False
            for ins in bb.instructions:
                si = ins.sync_info
                if si is not None and si.on_wait and len(si.on_wait) > 1:
                    waits = list(si.on_wait)
                    for i, w in enumerate(waits[:-1]):
                        new_insts.append(
                            mybir.InstNoOp(
                                name=f"{ins.name}-wsplit{i}",
                                engine=ins.engine,
                                sync_info=mybir.SyncInfo(on_wait=[w], on_update=[]),
                                bass_nofuse=True,
                            )
                        )
                    ins.sync_info = mybir.SyncInfo(
                        on_wait=[waits[-1]], on_update=list(si.on_update or [])
                    )
                    changed = True
                new_insts.append(ins)
            if changed:
                bb.instructions = new_insts


def _body(ctx, tc, io, flags):
    nc = tc.nc
    has_bq, has_bk, has_bv, has_gamma, has_beta = flags
    (x_d, xt_d, wqt_d, wkt_d, wvt_d, bq_d, bk_d, bv_d, gamma_d, beta_d,
     denmask_d, bcmask_d, ident_d, out_d) = io

    const = ctx.enter_context(tc.tile_pool(name="const", bufs=1))
    wpool = ctx.enter_context(tc.tile_pool(name="w", bufs=1))
    xtpool = ctx.enter_context(tc.tile_pool(name="xt", bufs=1))
    x2pool = ctx.enter_context(tc.tile_pool(name="x2", bufs=1))
    kvpool = ctx.enter_context(tc.tile_pool(name="kv", bufs=1))
    eqraw = ctx.enter_context(tc.tile_pool(name="eqr", bufs=2))
    eqpool = ctx.enter_context(tc.tile_pool(name="eq", bufs=1))
    opool = ctx.enter_context(tc.tile_pool(name="o", bufs=3))
    ypool = ctx.enter_context(tc.tile_pool(name="y", bufs=3))
    obpool = ctx.enter_context(tc.tile_pool(name="ob", bufs=3))
    jkpool = ctx.enter_context(tc.tile_pool(name="jk", bufs=2))
    gpool = ctx.enter_context(tc.tile_pool(name="g", bufs=1))
    smpool = ctx.enter_context(tc.tile_pool(name="sm", bufs=4))
    dram = ctx.enter_context(tc.tile_pool(name="dram", bufs=1, space="DRAM"))

    # PSUM: 8 banks of 2KB, allocated bufs-per-tag:
    #   ps_a: tag pk (sweep1 pk / q-phase pq)          2 banks
    #   ps_b: tag pv (sweep1 pv / q-phase pden+rqbc)   2 banks
    #   ps_g: tag pg (sweep1 per-pair G partials)      2 banks
    #   ps_o: tag po (sweep2 o-matmuls)                2 banks
    ps_a = ctx.enter_context(tc.tile_pool(name="ps_a", bufs=2, space="PSUM"))
    ps_b = ctx.enter_context(tc.tile_pool(name="ps_b", bufs=2, space="PSUM"))
    ps_g = ctx.enter_context(tc.tile_pool(name="ps_g", bufs=2, space="PSUM"))
    ps_o = ctx.enter_context(tc.tile_pool(name="ps_o", bufs=2, space="PSUM"))

    # ---- startup DMAs (critical first: wk, wv, xt0) ---------------------
    wq = wpool.tile([P, NK2, 2, DM], F8, name="wq")
    wk = wpool.tile([P, NK2, 2, DM], F8, name="wk")
    wv = wpool.tile([P, NK2, 2, DM], F8, name="wv")
    xt = [xtpool.tile([P, NK2, 2, CHUNK], F8, tag=f"xt{c}", name=f"xt{c}")
          for c in range(NCHUNK)]
    for t2 in range(NK2):
        nc.sync.dma_start(out=xt[0][:, t2], in_=xt_d[0, :, t2])
        nc.scalar.dma_start(out=wk[:, t2], in_=wkt_d[:, t2])
        nc.gpsimd.dma_start(out=wv[:, t2], in_=wvt_d[:, t2])
    nc.sync.dma_start(out=xt[1][:], in_=xt_d[1])
    nc.gpsimd.dma_start(out=xt[2][:], in_=xt_d[2])
    nc.sync.dma_start(out=xt[3][:], in_=xt_d[3])
    # wq only needed for the q phase - issue after wk on the same queue
    for t2 in range(NK2):
        nc.scalar.dma_start(out=wq[:, t2], in_=wqt_d[:, t2])
    # residual x blocks: prefetch ALL of them during sweep 1 (idle DMA)
    x_tiles = []
    for b in range(NBLK):
        x_b = x2pool.tile([P, DM], BF16, tag=f"x2b{b}", name=f"x2b{b}")
        nc.sync.dma_start(out=x_b[:], in_=x_d[b * P:(b + 1) * P, :])
        x_tiles.append(x_b)

    # ---- constants ------------------------------------------------------
    eps_t = const.tile([P, 1], F32)
    nc.vector.memset(eps_t[:], EPS)

    # den masks: maskm[ch, m, h] = 1 iff h == 2m + ch//64.  Each m-tile's
    # matmul writes the full [16, 512] denominator tile (zero elsewhere)
    # so all 8 accumulate into one PSUM group at base partition 0.
    # broadcast masks: bcm[h(16 part), m, ch(128)] = same predicate.
    # Host-prepared (partition-odd memsets are not encodable).
    maskm = const.tile([P, NKT, H], F8)
    nc.scalar.dma_start(out=maskm[:], in_=denmask_d)
    bcm = const.tile([16, NKT, P], BF16)
    nc.scalar.dma_start(out=bcm[:], in_=bcmask_d)
    ident = const.tile([P, P], BF16)
    nc.scalar.dma_start(out=ident[:], in_=ident_d)

    bq_t = None
    if has_bq:
        bq_t = const.tile([P, NKT], F32)
        nc.sync.dma_start(out=bq_t[:], in_=bq_d.rearrange("(t p) -> p t", p=P))
    bk_bc = bv_bc = gamma_bc = beta_bc = None

    def _bcast(src_d):
        t = const.tile([P, DM], F32, name=f"bc_{src_d.tensor.name}")
        src = bass.AP(tensor=src_d.tensor, offset=src_d.offset,
                      ap=[[0, P]] + list(src_d.ap))
        nc.sync.dma_start(out=t[:], in_=src)
        return t

    if has_bk:
        bk_bc = _bcast(bk_d)
        nc.vector.tensor_scalar_mul(out=bk_bc[:], in0=bk_bc[:], scalar1=SW)
    if has_bv:
        bv_bc = _bcast(bv_d)
        nc.vector.tensor_scalar_mul(out=bv_bc[:], in0=bv_bc[:], scalar1=SW)
    if has_gamma:
        gamma_bc = _bcast(gamma_d)
    if has_beta:
        beta_bc = _bcast(beta_d)

    # ---- sweep-1 SBUF tiles --------------------------------------------
    # expk2[i]: [row, half, ch]; vext[i]: [row, half, pair, 130] (cols
    # 128:130 are the ones columns -> colsum).  Two tiles alternate
    # across block-pairs; ones memset once.
    expk2 = [kvpool.tile([P, 2, DM], F8, tag=f"ek{i}", name=f"ek{i}")
             for i in range(2)]
    vext = [kvpool.tile([P, 2, NPAIR, 130], F8, tag=f"vx{i}", name=f"vx{i}")
            for i in range(2)]
    nc.vector.memset(vext[0][:, :, :, 128:130], 1.0)
    nc.vector.memset(vext[1][:, :, :, 128:130], 1.0)

    # G accumulator in SBUF: [row-ch, pair, 130], col 128 = colsum
    gacc = gpool.tile([P, NPAIR, 130], F32, name="gacc")
    nc.vector.memset(gacc[:], 0.0)

    # ============ sweep 1: k/v projections + G accumulation =============
    def _emit_kv(b, ek, vx, half):
        c, j = divmod(b, BPC)
        js = slice(j * P, (j + 1) * P)
        for cc in range(2):
            cs = slice(cc * CHUNK, (cc + 1) * CHUNK)
            pk = ps_a.tile([P, CHUNK], F32, tag="pk", name="pk")
            pv = ps_b.tile([P, CHUNK], F32, tag="pv", name="pv")
            for t2 in range(NK2):
                lhsT = xt[c][:, t2, :, js]
                nc.tensor.matmul(pk[:], lhsT, wk[:, t2, :, cs], perf_mode=DR,
                                 start=(t2 == 0), stop=(t2 == NK2 - 1))
                nc.tensor.matmul(pv[:], lhsT, wv[:, t2, :, cs], perf_mode=DR,
                                 start=(t2 == 0), stop=(t2 == NK2 - 1))
            # exp(k) eviction (ACT), unscale folded
            if has_bk:
                nc.vector.tensor_add(out=pk[:], in0=pk[:], in1=bk_bc[:, cs])
            nc.scalar.activation(out=ek[:, half, cs], in_=pk[:],
                                 func=AF.Exp, scale=ISW)
            # v eviction (DVE)
            if has_bv:
                nc.vector.tensor_add(out=pv[:], in0=pv[:], in1=bv_bc[:, cs])
            vdst = vx[:, half, 4 * cc:4 * (cc + 1), 0:128]
            psrc = pv[:].rearrange("p (a b) -> p a b", a=4)
            nc.vector.tensor_scalar_mul(out=vdst, in0=psrc, scalar1=ISW)

    def _emit_g(ek, vx, bp):
        # G += expk_pair.T @ [v_pair | 1 | 1] (DoubleRow over the 2
        # stacked blocks); 2 pairs share one PSUM bank, accumulation
        # across block-pairs on the DVE.
        for i in range(NPAIR // 2):
            pg = ps_g.tile([P, 2, 130], F32, tag="pg", name="pg")
            for u in range(2):
                p = 2 * i + u
                nc.tensor.matmul(pg[:, u, :], ek[:, :, p * P:(p + 1) * P],
                                 vx[:, :, p, :], perf_mode=DR,
                                 start=True, stop=True)
            nc.vector.tensor_add(out=gacc[:, 2 * i:2 * i + 2, :],
                                 in0=gacc[:, 2 * i:2 * i + 2, :], in1=pg[:])

    pending = None
    for bp in range(NBLK // 2):
        ek, vx = expk2[bp % 2], vext[bp % 2]
        _emit_kv(2 * bp, ek, vx, 0)
        if pending is not None:
            _emit_g(*pending)
        _emit_kv(2 * bp + 1, ek, vx, 1)
        pending = (ek, vx, bp)
    _emit_g(*pending)

    # ================= packed AllReduce of G ============================
    # pack [128, pair, 130] -> [128, pair, 65]: only the two diagonal
    # 64x64 head blocks + the colsum column survive (halves the payload)
    gsb = gpool.tile([P, NPAIR, 65], F32, name="gsb")
    nc.vector.tensor_copy(out=gsb[0:64, :, 0:64], in_=gacc[0:64, :, 0:64])
    nc.vector.tensor_copy(out=gsb[64:128, :, 0:64],
                          in_=gacc[64:128, :, 64:128])
    nc.vector.tensor_copy(out=gsb[:, :, 64:65], in_=gacc[:, :, 128:129])
    g_in = dram.tile([P, NPAIR, 65], F32)
    g_out = dram.tile([P, NPAIR, 65], F32)
    nc.sync.dma_start(out=g_in[:], in_=gsb[:])
    nc.gpsimd.collective_compute(
        "AllReduce", ALU.add,
        replica_groups=[[0, 1], [2, 3], [4, 5], [6, 7]],
        ins=[g_in.opt()], outs=[g_out.opt()],
    )
    gall = gpool.tile([P, NPAIR, 65], F32, name="gall")
    nc.sync.dma_start(out=gall[:], in_=g_out[:])

    # ============ q phase: projection + softmax normalization ===========
    # eq_n[c]: fp8 [ch, m, row] = exp(q)/den, fully normalized.
    eq_n = [eqpool.tile([P, NKT, CHUNK], F8, tag=f"eqn{c}", name=f"eqn{c}")
            for c in range(NCHUNK)]
    rqT = gpool.tile([16, CHUNK], BF16, name="rqT")

    for cb in range(NCHUNK):
        er = eqraw.tile([P, NKT, CHUNK], F8, tag="eqr", name="eqr")
        for m in range(NKT):
            pq = ps_a.tile([P, CHUNK], F32, tag="pk", name="pq")
            for t2 in range(NK2):
                nc.tensor.matmul(pq[:], wq[:, t2, :, m * P:(m + 1) * P],
                                 xt[cb][:, t2], perf_mode=DR,
                                 start=(t2 == 0), stop=(t2 == NK2 - 1))
            if has_bq:
                nc.scalar.activation(out=er[:, m, :], in_=pq[:], func=AF.Exp,
                                     bias=bq_t[:, m:m + 1], scale=ISW)
            else:
                nc.scalar.activation(out=er[:, m, :], in_=pq[:], func=AF.Exp,
                                     scale=ISW)
        # denominators: pden.T[h, row] = sum over head h's channels
        # (each m-tile matmul writes all 16 head rows, zeros elsewhere,
        # accumulating in PSUM)
        pden = ps_b.tile([16, CHUNK], F32, tag="pv", name="pden")
        for m in range(NKT):
            nc.tensor.matmul(pden[:], maskm[:, m, :], er[:, m, :],
                             start=(m == 0), stop=(m == NKT - 1))
        # reciprocal -> bf16 [16, 512] (DVE; ACT recip is blocked)
        with nc.allow_low_precision(reason="1/den at bf16: 0.4% on softmax "
                                    "weights vs 2e-2 output tolerance"):
            nc.vector.reciprocal(out=rqT[:], in_=pden[:])
        # normalize eq per m-tile: PE broadcast [16,512]->[128,512], DVE mul
        for m in range(NKT):
            rqbc = ps_b.tile([P, CHUNK], F32, tag="pv", name="rqbc")
            nc.tensor.matmul(rqbc[:], bcm[:, m, :], rqT[:],
                             start=True, stop=True)
            nc.vector.tensor_mul(out=eq_n[cb][:, m, :], in0=er[:, m, :],
                                 in1=rqbc[:])

    # ---- g2: block-diagonal per-pair G/colsum, fp8, pair-stacked -------
    # g2[k, i, u, n]: pair p = 2u+i occupies cols 128*i..128*i+128.
    rcs = gpool.tile([P, NPAIR], F32, name="rcs")
    nc.vector.reciprocal(out=rcs[:], in_=gall[:, :, 64])
    g2 = gpool.tile([P, 2, 4, 2 * P], F8, name="g2")
    nc.vector.memset(g2[:], 0.0)
    for p in range(NPAIR):
        i, u = p % 2, p // 2
        base = P * i
        nc.vector.tensor_scalar_mul(
            out=g2[0:64, i, u, base:base + 64],
            in0=gall[0:64, p, 0:64], scalar1=rcs[0:64, p:p + 1])
        nc.vector.tensor_scalar_mul(
            out=g2[64:128, i, u, base + 64:base + 128],
            in0=gall[64:128, p, 0:64], scalar1=rcs[64:128, p:p + 1])

    # ====== sweep 2: o matmuls + residual + LayerNorm ===================
    # Per block: 4 pair-stacked DR o-matmuls (N=256) + one identity
    # matmul per po tile folds the residual x into PSUM, so po IS
    # y = x + o.  ACT evicts y (bf16) with accum_out giving sum(y) for
    # free; DVE does sum(y^2) + the normalize; the tiny LN scalar chain
    # is batched across 4-block groups.
    GB = 4  # blocks per stats group
    for bg in range(NBLK // GB):
        blocks = range(bg * GB, (bg + 1) * GB)
        syh = smpool.tile([P, GB, 2], F32, tag="syh", name="syh")
        sy2 = smpool.tile([P, GB], F32, tag="sy2", name="sy2")
        ybs = []
        for b in blocks:
            c, j = divmod(b, BPC)
            js = slice(j * P, (j + 1) * P)
            bi = b % GB
            po = [None, None]
            for t in range(2):
                pot = ps_o.tile([P, 2 * 2 * P], F32, tag="po", name="po")
                for uu in range(2):
                    u = 2 * t + uu
                    nc.tensor.matmul(pot[:, 256 * uu:256 * (uu + 1)],
                                     eq_n[c][:, 2 * u:2 * u + 2, js],
                                     g2[:, :, u, :], perf_mode=DR,
                                     start=True, stop=False,
                                     skip_group_check=True)
                # accumulate the residual: po += I.T @ x_half
                nc.tensor.matmul(pot[:], ident[:],
                                 x_tiles[b][:, 512 * t:512 * (t + 1)],
                                 start=False, stop=True,
                                 skip_group_check=True)
                po[t] = pot
            # ACT evicts y (bf16) + per-half row sums
            y_b = ypool.tile([P, DM], BF16, tag=f"yb{bi}", name="yb", bufs=2)
            nc.scalar.activation(out=y_b[:, 0:512], in_=po[0][:],
                                 func=AF.Identity,
                                 accum_out=syh[:, bi, 0:1])
            nc.scalar.activation(out=y_b[:, 512:1024], in_=po[1][:],
                                 func=AF.Identity,
                                 accum_out=syh[:, bi, 1:2])
            # sum(y^2) on DVE (one pass, bf16 2x mode)
            junk = jkpool.tile([P, DM], BF16, tag="jk", name="jk")
            nc.vector.scalar_tensor_tensor(
                out=junk[:], in0=y_b[:], scalar=0.0, in1=y_b[:],
                op0=ALU.add, op1=ALU.mult, accum_out=sy2[:, bi:bi + 1])
            ybs.append(y_b)
        # batched LN scalar chain for the group: mu, var, rstd, bias
        sy = smpool.tile([P, GB], F32, tag="sy", name="sy")
        nc.vector.tensor_add(out=sy[:], in0=syh[:, :, 0], in1=syh[:, :, 1])
        mu = smpool.tile([P, GB], F32, tag="mu", name="mu")
        nc.vector.tensor_scalar_mul(out=mu[:], in0=sy[:], scalar1=INV_N)
        num = smpool.tile([P, GB], F32, tag="num", name="num")
        nc.vector.tensor_mul(out=num[:], in0=sy[:], in1=mu[:])
        nc.vector.tensor_sub(out=num[:], in0=num[:], in1=sy2[:])
        rstd = smpool.tile([P, 2, GB], F32, tag="rstd", name="rstd")
        nc.scalar.activation(out=rstd[:, 0, :], in_=num[:], func=AF.Sqrt,
                             scale=-INV_N, bias=eps_t[:])
        nc.vector.reciprocal(out=rstd[:, 1, :], in_=rstd[:, 0, :])
        nbias = smpool.tile([P, GB], F32, tag="nb", name="nb")
        nc.vector.scalar_tensor_tensor(out=nbias[:], in0=mu[:], scalar=-1.0,
                                       in1=rstd[:, 1, :], op0=ALU.mult,
                                       op1=ALU.mult)
        # normalize + store
        for b in blocks:
            bi = b % GB
            y_b = ybs[bi]
            ob = obpool.tile([P, DM], BF16, tag="oo", name="oo")
            if has_gamma or has_beta:
                yn = ypool.tile([P, DM], F32, tag="yn", name="yn", bufs=2)
                nc.vector.tensor_scalar(out=yn[:], in0=y_b[:],
                                        scalar1=rstd[:, 1, bi:bi + 1],
                                        scalar2=nbias[:, bi:bi + 1],
                                        op0=ALU.mult, op1=ALU.add)
                if has_gamma:
                    nc.vector.tensor_mul(out=yn[:], in0=yn[:], in1=gamma_bc[:])
                if has_beta:
                    nc.vector.tensor_add(out=ob[:], in0=yn[:], in1=beta_bc[:])
                else:
                    nc.vector.tensor_copy(out=ob[:], in_=yn[:])
            else:
                nc.vector.tensor_scalar(out=ob[:], in0=y_b[:],
                                        scalar1=rstd[:, 1, bi:bi + 1],
                                        scalar2=nbias[:, bi:bi + 1],
                                        op0=ALU.mult, op1=ALU.add)
            nc.sync.dma_start(out=out_d[b * P:(b + 1) * P, :], in_=ob[:])


_PROGRAM_CACHE = {}


def _build_program(flags):
    if flags in _PROGRAM_CACHE:
        return _PROGRAM_CACHE[flags]
    nc = bass.Bass("TRN2", target_bir_lowering=False, debug=False,
                   num_devices=NCORES)
    x_d = nc.dram_tensor("x_shard", [R, DM], BF16, kind="ExternalInput").ap()
    xt_d = nc.dram_tensor("xt8", [NCHUNK, P, NK2, 2, CHUNK], F8,
                          kind="ExternalInput").ap()
    wqt_d = nc.dram_tensor("wq8", [P, NK2, 2, DM], F8, kind="ExternalInput").ap()
    wkt_d = nc.dram_tensor("wk8", [P, NK2, 2, DM], F8, kind="ExternalInput").ap()
    wvt_d = nc.dram_tensor("wv8", [P, NK2, 2, DM], F8, kind="ExternalInput").ap()
    bq_d = nc.dram_tensor("bq", [DM], F32, kind="ExternalInput").ap()
    bk_d = nc.dram_tensor("bk", [DM], F32, kind="ExternalInput").ap()
    bv_d = nc.dram_tensor("bv", [DM], F32, kind="ExternalInput").ap()
    gamma_d = nc.dram_tensor("gamma", [DM], F32, kind="ExternalInput").ap()
    beta_d = nc.dram_tensor("beta", [DM], F32, kind="ExternalInput").ap()
    denmask_d = nc.dram_tensor("denmask", [P, NKT, H], F8,
                               kind="ExternalInput").ap()
    bcmask_d = nc.dram_tensor("bcmask", [16, NKT, P], BF16,
                              kind="ExternalInput").ap()
    ident_d = nc.dram_tensor("ident", [P, P], BF16, kind="ExternalInput").ap()
    out_d = nc.dram_tensor("out_shard", [R, DM], BF16, kind="ExternalOutput").ap()
    io = (x_d, xt_d, wqt_d, wkt_d, wvt_d, bq_d, bk_d, bv_d, gamma_d, beta_d,
          denmask_d, bcmask_d, ident_d, out_d)
    with tile.TileContext(nc) as tc:
        with ExitStack() as ctx:
            _body(ctx, tc, io, flags)
    _fix_multiwaits(nc)
    _PROGRAM_CACHE[flags] = nc
    return nc


def _prep_inputs(x, Wq, bq, Wk, bk, Wv, bv, gamma, beta):
    """Host-side: shard x, build fp8 layouts. Returns (in_maps, flags)."""
    import ml_dtypes
    f8 = ml_dtypes.float8_e4m3
    bf16 = ml_dtypes.bfloat16
    x = np.ascontiguousarray(np.asarray(x, dtype=np.float32))
    flags = (bool(np.any(bq)), bool(np.any(bk)), bool(np.any(bv)),
             bool(np.any(np.asarray(gamma) != 1.0)), bool(np.any(beta)))

    def _w8(W):
        # [P, NK2, 2, DM]: [p,t2,i,n] = SW * W[n, (2t2+i)*128+p]
        Wt = (np.asarray(W, dtype=np.float32).T * SW).astype(f8)  # [in, out]
        return np.ascontiguousarray(
            Wt.reshape(NK2, 2, P, DM).transpose(2, 0, 1, 3))

    # masks: predicate h == 2m + ch//64
    ch = np.arange(P)
    mm_, hh = np.meshgrid(np.arange(NKT), np.arange(H), indexing="ij")
    pred = (hh[None, :, :] == 2 * mm_[None, :, :] + (ch[:, None, None] // 64))
    denmask = pred.astype(np.float32).astype(f8)               # [P, NKT, H]
    bcmask = np.ascontiguousarray(
        pred.transpose(2, 1, 0).astype(np.float32).astype(bf16))  # [H,NKT,P]

    common = {
        "wq8": _w8(Wq), "wk8": _w8(Wk), "wv8": _w8(Wv),
        "denmask": np.ascontiguousarray(denmask),
        "bcmask": bcmask,
        "ident": np.eye(P, dtype=np.float32).astype(bf16),
        "bq": np.ascontiguousarray(bq, dtype=np.float32),
        "bk": np.ascontiguousarray(bk, dtype=np.float32),
        "bv": np.ascontiguousarray(bv, dtype=np.float32),
        "gamma": np.ascontiguousarray(gamma, dtype=np.float32),
        "beta": np.ascontiguousarray(beta, dtype=np.float32),
    }
    in_maps = []
    for c in range(NCORES):
        b, half = divmod(c, 2)
        shard = np.ascontiguousarray(x[b, half * R:(half + 1) * R, :])
        # xt8 [NCHUNK, P, NK2, 2, CHUNK]: [c,p,t2,i,r] = x[c*512+r, (2t2+i)*128+p]
        x8 = shard.astype(f8).reshape(NCHUNK, CHUNK, NK2, 2, P)
        x8 = np.ascontiguousarray(x8.transpose(0, 4, 2, 3, 1))
        in_maps.append({"x_shard": shard.astype(bf16), "xt8": x8, **common})
    return in_maps, flags


def kernel(x, mask, pad_mask, Wq, bq, Wk, bk, Wv, bv, gamma, beta):
    in_maps, flags = _prep_inputs(x, Wq, bq, Wk, bk, Wv, bv, gamma, beta)
    nc = _build_program(flags)
    res = run_bass_kernel_spmd(nc, in_maps, list(range(NCORES)))
    out = np.empty((B, S, DM), dtype=np.float32)
    for c in range(NCORES):
        b, half = divmod(c, 2)
        out[b, half * R:(half + 1) * R, :] = np.asarray(
            res.results[c]["out_shard"]).astype(np.float32)
    return out


if __name__ == "__main__":
    rng = np.random.default_rng(0)
    s = 1.0 / np.sqrt(DM)
    demo = {
        "x": rng.standard_normal((B, S, DM), dtype=np.float32),
        "mask": np.zeros((S, S), bool),
        "pad_mask": np.zeros((B, S), bool),
        "Wq": rng.uniform(-s, s, (DM, DM)).astype(np.float32),
        "bq": np.zeros(DM, np.float32),
        "Wk": rng.uniform(-s, s, (DM, DM)).astype(np.float32),
        "bk": np.zeros(DM, np.float32),
        "Wv": rng.uniform(-s, s, (DM, DM)).astype(np.float32),
        "bv": np.zeros(DM, np.float32),
        "gamma": np.ones(DM, np.float32),
        "beta": np.zeros(DM, np.float32),
    }
    out = kernel(**demo)
    print("out", out.shape, out.dtype, float(np.abs(out).max()))


# revision 34
# speedup vs baseline: 1.0477x; 1.0477x over previous
"""Trainium2 Bass kernel for nn_MultiHeadAttention_Linear_11312943857747.

Math (B=4, S=4096, DM=1024, H=16, HD=64):
    q = softmax(x @ Wq.T + bq) over head_dim
    k = softmax(x @ Wk.T + bk) over seq_len
    v = x @ Wv.T + bv
    gmap[b,h] = k[b,h].T @ v[b,h]            (HD x HD per head)
    o[b,h]    = q[b,h] @ gmap[b,h]
    out = LayerNorm(x + o) * gamma + beta

Sharding: 8 cores = 4 batches x 2 sequence-halves. Each core projects its
2048 rows; the per-head kT@v reduction over the full sequence is completed
with a packed 266KB AllReduce between the two cores sharing a batch
(replica groups [[0,1],[2,3],[4,5],[6,7]]).

Both softmaxes fold into matmuls; exp() needs no max-subtraction
(|q|,|k| <~ 4, softmax is shift-invariant).  Projections run fp8e4 with
DoubleRow (256-deep contraction, weights host-scaled by SW=256; the 1/SW
unscale folds into the exp() activation scale and the v-eviction).

Schedule:
  sweep 1: k/v projections per block; per-head G matmuls (col-tiled,
           DoubleRow over block pairs) accumulate directly in PSUM across
           all 8 block-pairs - no vector adds.  x residual blocks prefetch
           on the idle DMA queue throughout.
  G is copied packed [128,8,65] (only diagonal 64x64 blocks + colsum) and
  AllReduced while the whole q phase runs.
  q phase (per 512-row chunk): q projection -> exp; per-head softmax
           denominators via a mask-stationary matmul into [16,512] PSUM;
           ACT reciprocal; PE broadcast back to [128,512]; DVE scales
           exp(q) by 1/den -> eq_n (fp8).  All softmax normalization of q
           is finished here, hidden under the projection matmul stream.
  sweep 2: per block: 4 pair-stacked DoubleRow o-matmuls (N=256) of
           eq_n @ g2 (g2 = G/colsum, block-diagonal, fp8); ACT evicts
           po -> o (bf16); DVE adds the residual with accum_out giving
           sum(y) for free; GpSimd computes sum(y^2); LN closes with an
           ACT rsqrt + one DVE tensor_scalar (4x mode) normalize.
"""

import sys

sys.path.insert(0, "/opt/trn_rl_repo")

import numpy as np
from contextlib import ExitStack

import concourse.bass as bass
import concourse.mybir as mybir
import concourse.tile as tile
from concourse.bass_utils import run_bass_kernel_spmd

F32 = mybir.dt.float32
BF16 = mybir.dt.bfloat16
F8 = mybir.dt.float8e4
DR = mybir.MatmulPerfMode.DoubleRow
AF = mybir.ActivationFunctionType
ALU = mybir.AluOpType

B, S, DM, H, HD = 4, 4096, 1024, 16, 64
EPS = 1e-5
NCORES = 8
R = S // 2          # rows per core
P = 128             # partitions
NBLK = R // P       # 16 sequence blocks of 128 rows
NKT = DM // P       # 8 k-tiles over the contraction dim
NK2 = NKT // 2      # 4 double-k-tiles (256 contraction per DoubleRow pass)
NPAIR = DM // P     # 8 head-pairs (2 heads of 64 = 128 channels)
CHUNK = 512         # moving-operand width for the big projections
NCHUNK = R // CHUNK # 4
BPC = CHUNK // P    # 4 blocks per chunk
SW = 256.0          # host-side weight scale for fp8 range
ISW = 1.0 / SW
INV_N = 1.0 / DM


def _fix_multiwaits(nc):
    """This walrus build encodes at most one sync wait per instruction;
    split any multi-wait instruction into preceding same-engine NoOps."""
    for fn in nc.m.functions:
        for bb in fn.blocks:
            new_insts = []
            changed = False
            for ins in bb.instructions:
                si = ins.sync_info
                if si is not None and si.on_wait and len(si.on_wait) > 1:
                    waits = list(si.on_wait)
                    for i, w in enumerate(waits[:-1]):
                        new_insts.append(
                            mybir.InstNoOp(
                                name=f"{ins.name}-wsplit{i}",
                                engine=ins.engine,
                                sync_info=mybir.SyncInfo(on_wait=[w], on_update=[]),
                                bass_nofuse=True,
                            )
                        )
                    ins.sync_info = mybir.SyncInfo(
                        on_wait=[waits[-1]], on_update=list(si.on_update or [])
                    )
                    changed = True
                new_insts.append(ins)
            if changed:
                bb.instructions = new_insts


def _body(ctx, tc, io, flags):
    nc = tc.nc
    has_bq, has_bk, has_bv, has_gamma, has_beta = flags
    (x_d, xt_d, wqt_d, wkt_d, wvt_d, bq_d, bk_d, bv_d, gamma_d, beta_d,
     denmask_d, bcmask_d, ident_d, out_d) = io

    const = ctx.enter_context(tc.tile_pool(name="const", bufs=1))
    wpool = ctx.enter_context(tc.tile_pool(name="w", bufs=1))
    xtpool = ctx.enter_context(tc.tile_pool(name="xt", bufs=1))
    x2pool = ctx.enter_context(tc.tile_pool(name="x2", bufs=1))
    kvpool = ctx.enter_context(tc.tile_pool(name="kv", bufs=1))
    eqraw = ctx.enter_context(tc.tile_pool(name="eqr", bufs=2))
    eqpool = ctx.enter_context(tc.tile_pool(name="eq", bufs=1))
    opool = ctx.enter_context(tc.tile_pool(name="o", bufs=3))
    ypool = ctx.enter_context(tc.tile_pool(name="y", bufs=3))
    obpool = ctx.enter_context(tc.tile_pool(name="ob", bufs=3))
    jkpool = ctx.enter_context(tc.tile_pool(name="jk", bufs=2))
    gpool = ctx.enter_context(tc.tile_pool(name="g", bufs=1))
    smpool = ctx.enter_context(tc.tile_pool(name="sm", bufs=4))
    dram = ctx.enter_context(tc.tile_pool(name="dram", bufs=1, space="DRAM"))

    # PSUM: 8 banks of 2KB, allocated bufs-per-tag:
    #   ps_a: tag pk (sweep1 pk / q-phase pq)          2 banks
    #   ps_b: tag pv (sweep1 pv / q-phase pden+rqbc)   2 banks
    #   ps_g: tag pg (sweep1 per-pair G partials)      2 banks
    #   ps_o: tag po (sweep2 o-matmuls)                2 banks
    ps_a = ctx.enter_context(tc.tile_pool(name="ps_a", bufs=2, space="PSUM"))
    ps_b = ctx.enter_context(tc.tile_pool(name="ps_b", bufs=2, space="PSUM"))
    ps_g = ctx.enter_context(tc.tile_pool(name="ps_g", bufs=2, space="PSUM"))
    ps_o = ctx.enter_context(tc.tile_pool(name="ps_o", bufs=2, space="PSUM"))

    # ---- startup DMAs (critical first: wk, wv, xt0) ---------------------
    wq = wpool.tile([P, NK2, 2, DM], F8, name="wq")
    wk = wpool.tile([P, NK2, 2, DM], F8, name="wk")
    wv = wpool.tile([P, NK2, 2, DM], F8, name="wv")
    xt = [xtpool.tile([P, NK2, 2, CHUNK], F8, tag=f"xt{c}", name=f"xt{c}")
          for c in range(NCHUNK)]
    for t2 in range(NK2):
        nc.sync.dma_start(out=xt[0][:, t2], in_=xt_d[0, :, t2])
        nc.scalar.dma_start(out=wk[:, t2], in_=wkt_d[:, t2])
        nc.gpsimd.dma_start(out=wv[:, t2], in_=wvt_d[:, t2])
    nc.sync.dma_start(out=xt[1][:], in_=xt_d[1])
    nc.gpsimd.dma_start(out=xt[2][:], in_=xt_d[2])
    nc.sync.dma_start(out=xt[3][:], in_=xt_d[3])
    # wq only needed for the q phase - issue after wk on the same queue
    for t2 in range(NK2):
        nc.scalar.dma_start(out=wq[:, t2], in_=wqt_d[:, t2])
    # residual x blocks: prefetch ALL of them during sweep 1 (idle DMA)
    x_tiles = []
    for b in range(NBLK):
        x_b = x2pool.tile([P, DM], BF16, tag=f"x2b{b}", name=f"x2b{b}")
        nc.sync.dma_start(out=x_b[:], in_=x_d[b * P:(b + 1) * P, :])
        x_tiles.append(x_b)

    # ---- constants ------------------------------------------------------
    eps_t = const.tile([P, 1], F32)
    nc.vector.memset(eps_t[:], EPS)

    # den masks: maskm[ch, m, h] = 1 iff h == 2m + ch//64.  Each m-tile's
    # matmul writes the full [16, 512] denominator tile (zero elsewhere)
    # so all 8 accumulate into one PSUM group at base partition 0.
    # broadcast masks: bcm[h(16 part), m, ch(128)] = same predicate.
    # Host-prepared (partition-odd memsets are not encodable).
    maskm = const.tile([P, NKT, H], F8)
    nc.scalar.dma_start(out=maskm[:], in_=denmask_d)
    bcm = const.tile([16, NKT, P], BF16)
    nc.scalar.dma_start(out=bcm[:], in_=bcmask_d)
    ident = const.tile([P, P], BF16)
    nc.scalar.dma_start(out=ident[:], in_=ident_d)

    bq_t = None
    if has_bq:
        bq_t = const.tile([P, NKT], F32)
        nc.sync.dma_start(out=bq_t[:], in_=bq_d.rearrange("(t p) -> p t", p=P))
    bk_bc = bv_bc = gamma_bc = beta_bc = None

    def _bcast(src_d):
        t = const.tile([P, DM], F32, name=f"bc_{src_d.tensor.name}")
        src = bass.AP(tensor=src_d.tensor, offset=src_d.offset,
                      ap=[[0, P]] + list(src_d.ap))
        nc.sync.dma_start(out=t[:], in_=src)
        return t

    if has_bk:
        bk_bc = _bcast(bk_d)
        nc.vector.tensor_scalar_mul(out=bk_bc[:], in0=bk_bc[:], scalar1=SW)
    if has_bv:
        bv_bc = _bcast(bv_d)
        nc.vector.tensor_scalar_mul(out=bv_bc[:], in0=bv_bc[:], scalar1=SW)
    if has_gamma:
        gamma_bc = _bcast(gamma_d)
    if has_beta:
        beta_bc = _bcast(beta_d)

    # ---- sweep-1 SBUF tiles --------------------------------------------
    # expk2[i]: [row, half, ch]; vext[i]: [row, half, pair, 130] (cols
    # 128:130 are the ones columns -> colsum).  Two tiles alternate
    # across block-pairs; ones memset once.
    expk2 = [kvpool.tile([P, 2, DM], F8, tag=f"ek{i}", name=f"ek{i}")
             for i in range(2)]
    vext = [kvpool.tile([P, 2, NPAIR, 130], F8, tag=f"vx{i}", name=f"vx{i}")
            for i in range(2)]
    nc.vector.memset(vext[0][:, :, :, 128:130], 1.0)
    nc.vector.memset(vext[1][:, :, :, 128:130], 1.0)

    # G accumulators in SBUF (one per sweep half, AllReduced separately):
    # [row-ch, pair, 130], col 128 = colsum
    gacc = [gpool.tile([P, NPAIR, 130], F32, name=f"gacc{i}")
            for i in range(2)]
    nc.vector.memset(gacc[0][:], 0.0)
    nc.vector.memset(gacc[1][:], 0.0)
    # g2 zero-fill early (only block-diagonals get written later)
    g2 = gpool.tile([P, 2, 4, 2 * P], F8, name="g2")
    nc.vector.memset(g2[:], 0.0)

    # ============ sweep 1: k/v projections + G accumulation =============
    def _emit_kv(b, ek, vx, half):
        c, j = divmod(b, BPC)
        js = slice(j * P, (j + 1) * P)
        for cc in range(2):
            cs = slice(cc * CHUNK, (cc + 1) * CHUNK)
            pk = ps_a.tile([P, CHUNK], F32, tag="pk", name="pk")
            pv = ps_b.tile([P, CHUNK], F32, tag="pv", name="pv")
            for t2 in range(NK2):
                lhsT = xt[c][:, t2, :, js]
                nc.tensor.matmul(pk[:], lhsT, wk[:, t2, :, cs], perf_mode=DR,
                                 start=(t2 == 0), stop=(t2 == NK2 - 1))
                nc.tensor.matmul(pv[:], lhsT, wv[:, t2, :, cs], perf_mode=DR,
                                 start=(t2 == 0), stop=(t2 == NK2 - 1))
            # exp(k) eviction (ACT), unscale folded
            if has_bk:
                nc.vector.tensor_add(out=pk[:], in0=pk[:], in1=bk_bc[:, cs])
            nc.scalar.activation(out=ek[:, half, cs], in_=pk[:],
                                 func=AF.Exp, scale=ISW)
            # v eviction (DVE)
            if has_bv:
                nc.vector.tensor_add(out=pv[:], in0=pv[:], in1=bv_bc[:, cs])
            vdst = vx[:, half, 4 * cc:4 * (cc + 1), 0:128]
            psrc = pv[:].rearrange("p (a b) -> p a b", a=4)
            nc.vector.tensor_scalar_mul(out=vdst, in0=psrc, scalar1=ISW)

    def _emit_g(ek, vx, bp):
        # G += expk_pair.T @ [v_pair | 1 | 1] (DoubleRow over the 2
        # stacked blocks); 2 pairs share one PSUM bank, accumulation
        # across block-pairs on the DVE.
        ga = gacc[bp // 4]
        for i in range(NPAIR // 2):
            pg = ps_g.tile([P, 2, 130], F32, tag="pg", name="pg")
            for u in range(2):
                p = 2 * i + u
                nc.tensor.matmul(pg[:, u, :], ek[:, :, p * P:(p + 1) * P],
                                 vx[:, :, p, :], perf_mode=DR,
                                 start=True, stop=True)
            nc.vector.tensor_add(out=ga[:, 2 * i:2 * i + 2, :],
                                 in0=ga[:, 2 * i:2 * i + 2, :], in1=pg[:])

    # -- split packed AllReduce: half h covers blocks 8h..8h+7 ----------
    # pack [128, pair, 130] -> bf16 [128, pair, 65]: only the diagonal
    # 64x64 head blocks + the colsum column survive (4x less payload
    # than the naive fp32 full-G exchange)
    gdram = [[dram.tile([P, NPAIR, 65], BF16, name=f"g{h}{d}")
              for d in range(2)] for h in range(2)]
    gred = [gpool.tile([P, NPAIR, 65], BF16, name=f"gred{h}")
            for h in range(2)]

    def _emit_allreduce(h):
        ga = gacc[h]
        gsb = gpool.tile([P, NPAIR, 65], BF16, tag="gsb", name="gsb", bufs=2)
        with nc.allow_low_precision(reason="G exchanged at bf16: 0.4% on a "
                                    "term that is <2% of the output"):
            nc.vector.tensor_copy(out=gsb[0:64, :, 0:64],
                                  in_=ga[0:64, :, 0:64])
            nc.vector.tensor_copy(out=gsb[64:128, :, 0:64],
                                  in_=ga[64:128, :, 64:128])
            nc.vector.tensor_copy(out=gsb[:, :, 64:65], in_=ga[:, :, 128:129])
        nc.sync.dma_start(out=gdram[h][0][:], in_=gsb[:])
        nc.gpsimd.collective_compute(
            "AllReduce", ALU.add,
            replica_groups=[[0, 1], [2, 3], [4, 5], [6, 7]],
            ins=[gdram[h][0].opt()], outs=[gdram[h][1].opt()],
        )
        nc.sync.dma_start(out=gred[h][:], in_=gdram[h][1][:])

    pending = None
    for bp in range(NBLK // 2):
        ek, vx = expk2[bp % 2], vext[bp % 2]
        _emit_kv(2 * bp, ek, vx, 0)
        if pending is not None:
            _emit_g(*pending)
            if pending[2] == 3:
                _emit_allreduce(0)
        _emit_kv(2 * bp + 1, ek, vx, 1)
        pending = (ek, vx, bp)
    _emit_g(*pending)
    _emit_allreduce(1)

    # ============ q phase: projection + softmax normalization ===========
    # eq_n[c]: fp8 [ch, m, row] = exp(q)/den, fully normalized.
    eq_n = [eqpool.tile([P, NKT, CHUNK], F8, tag=f"eqn{c}", name=f"eqn{c}")
            for c in range(NCHUNK)]

    def _emit_qchunk(cb):
        er = eqraw.tile([P, NKT, CHUNK], F8, tag="eqr", name="eqr")
        for m in range(NKT):
            pq = ps_a.tile([P, CHUNK], F32, tag="pk", name="pq")
            for t2 in range(NK2):
                nc.tensor.matmul(pq[:], wq[:, t2, :, m * P:(m + 1) * P],
                                 xt[cb][:, t2], perf_mode=DR,
                                 start=(t2 == 0), stop=(t2 == NK2 - 1))
            if has_bq:
                nc.scalar.activation(out=er[:, m, :], in_=pq[:], func=AF.Exp,
                                     bias=bq_t[:, m:m + 1], scale=ISW)
            else:
                nc.scalar.activation(out=er[:, m, :], in_=pq[:], func=AF.Exp,
                                     scale=ISW)
        # denominators: pden.T[h, row] = sum over head h's channels
        # (each m-tile matmul writes the full 16 head rows, zeros
        # elsewhere, accumulating in PSUM)
        pden = ps_b.tile([16, CHUNK], F32, tag="pv", name="pden")
        for m in range(NKT):
            nc.tensor.matmul(pden[:], maskm[:, m, :], er[:, m, :],
                             start=(m == 0), stop=(m == NKT - 1))
        # 1/den = exp(-ln(den)) on ACT (keeps the slow DVE reciprocal off
        # the q-phase critical path; LUT accuracy ~0.1% is plenty)
        lnd = smpool.tile([16, CHUNK], F32, tag="lnd", name="lnd", bufs=2)
        nc.scalar.activation(out=lnd[:], in_=pden[:], func=AF.Ln)
        rqT = smpool.tile([16, CHUNK], BF16, tag="rqT", name="rqT", bufs=2)
        nc.scalar.activation(out=rqT[:], in_=lnd[:], func=AF.Exp, scale=-1.0)
        # normalize eq per m-tile: PE broadcast [16,512]->[128,512], DVE mul
        for m in range(NKT):
            rqbc = ps_b.tile([P, CHUNK], F32, tag="pv", name="rqbc")
            nc.tensor.matmul(rqbc[:], bcm[:, m, :], rqT[:],
                             start=True, stop=True)
            nc.vector.tensor_mul(out=eq_n[cb][:, m, :], in0=er[:, m, :],
                                 in1=rqbc[:])

    def _emit_g2():
        # gall = AR(G_half0) + AR(G_half1); g2 = block-diagonal G/colsum
        # (fp8, pair-stacked).  The 16 scale ops run on ACT (per-partition
        # scale AP) to keep the DVE free for the q-phase normalizations.
        gall = gpool.tile([P, NPAIR, 65], F32, name="gall")
        nc.vector.tensor_add(out=gall[:], in0=gred[0][:], in1=gred[1][:])
        rcs = gpool.tile([P, NPAIR], F32, name="rcs")
        nc.vector.reciprocal(out=rcs[:], in_=gall[:, :, 64])
        for p in range(NPAIR):
            i, u = p % 2, p // 2
            base = P * i
            nc.scalar.activation(out=g2[0:64, i, u, base:base + 64],
                                 in_=gall[0:64, p, 0:64], func=AF.Identity,
                                 scale=rcs[0:64, p:p + 1])
            nc.scalar.activation(out=g2[64:128, i, u, base + 64:base + 128],
                                 in_=gall[64:128, p, 0:64], func=AF.Identity,
                                 scale=rcs[64:128, p:p + 1])

    # ====== sweep 2: o matmuls + residual + LayerNorm ===================
    # Per block: 4 pair-stacked DR o-matmuls (N=256) + one identity
    # matmul per po tile folds the residual x into PSUM, so po IS
    # y = x + o.  ACT evicts y (bf16) with accum_out giving sum(y) for
    # free; DVE does sum(y^2) + the normalize; the tiny LN scalar chain
    # is batched per 4-block chunk and emitted one chunk late (software
    # pipelining) so the ACT sqrt never stalls the eviction stream.
    GB = BPC  # stats group == chunk
    s2state = {}

    def _emit_s2_mm(c):
        blocks = range(c * GB, (c + 1) * GB)
        syh = smpool.tile([P, GB, 2], F32, tag=f"syh{c % 2}", name="syh")
        sy2 = smpool.tile([P, GB], F32, tag=f"sy2{c % 2}", name="sy2")
        ybs = []
        for b in blocks:
            j = b % BPC
            js = slice(j * P, (j + 1) * P)
            bi = b % GB
            po = [None, None]
            for t in range(2):
                pot = ps_o.tile([P, 2 * 2 * P], F32, tag="po", name="po")
                for uu in range(2):
                    u = 2 * t + uu
                    nc.tensor.matmul(pot[:, 256 * uu:256 * (uu + 1)],
                                     eq_n[c][:, 2 * u:2 * u + 2, js],
                                     g2[:, :, u, :], perf_mode=DR,
                                     start=True, stop=False,
                                     skip_group_check=True)
                # accumulate the residual: po += I.T @ x_half
                nc.tensor.matmul(pot[:], ident[:],
                                 x_tiles[b][:, 512 * t:512 * (t + 1)],
                                 start=False, stop=True,
                                 skip_group_check=True)
                po[t] = pot
            # ACT evicts y (bf16) + per-half row sums
            y_b = ypool.tile([P, DM], BF16, tag=f"yb{bi}", name="yb", bufs=2)
            nc.scalar.activation(out=y_b[:, 0:512], in_=po[0][:],
                                 func=AF.Identity,
                                 accum_out=syh[:, bi, 0:1])
            nc.scalar.activation(out=y_b[:, 512:1024], in_=po[1][:],
                                 func=AF.Identity,
                                 accum_out=syh[:, bi, 1:2])
            # sum(y^2) on DVE
            junk = jkpool.tile([P, DM], BF16, tag="jk", name="jk")
            nc.vector.scalar_tensor_tensor(
                out=junk[:], in0=y_b[:], scalar=0.0, in1=y_b[:],
                op0=ALU.add, op1=ALU.mult, accum_out=sy2[:, bi:bi + 1])
            ybs.append(y_b)
        s2state[c] = (syh, sy2, ybs)

    def _emit_s2_stats(c):
        syh, sy2, ybs = s2state.pop(c)
        blocks = range(c * GB, (c + 1) * GB)
        # batched LN scalar chain for the group: mu, var, rstd, bias
        sy = smpool.tile([P, GB], F32, tag="sy", name="sy")
        nc.vector.tensor_add(out=sy[:], in0=syh[:, :, 0], in1=syh[:, :, 1])
        mu = smpool.tile([P, GB], F32, tag="mu", name="mu")
        nc.vector.tensor_scalar_mul(out=mu[:], in0=sy[:], scalar1=INV_N)
        num = smpool.tile([P, GB], F32, tag="num", name="num")
        nc.vector.tensor_mul(out=num[:], in0=sy[:], in1=mu[:])
        nc.vector.tensor_sub(out=num[:], in0=num[:], in1=sy2[:])
        rstd = smpool.tile([P, 2, GB], F32, tag="rstd", name="rstd")
        nc.scalar.activation(out=rstd[:, 0, :], in_=num[:], func=AF.Sqrt,
                             scale=-INV_N, bias=eps_t[:])
        nc.vector.reciprocal(out=rstd[:, 1, :], in_=rstd[:, 0, :])
        nbias = smpool.tile([P, GB], F32, tag="nb", name="nb")
        nc.vector.scalar_tensor_tensor(out=nbias[:], in0=mu[:], scalar=-1.0,
                                       in1=rstd[:, 1, :], op0=ALU.mult,
                                       op1=ALU.mult)
        # normalize + store
        for b in blocks:
            bi = b % GB
            y_b = ybs[bi]
            ob = obpool.tile([P, DM], BF16, tag="oo", name="oo")
            if has_gamma or has_beta:
                yn = ypool.tile([P, DM], F32, tag="yn", name="yn", bufs=2)
                nc.vector.tensor_scalar(out=yn[:], in0=y_b[:],
                                        scalar1=rstd[:, 1, bi:bi + 1],
                                        scalar2=nbias[:, bi:bi + 1],
                                        op0=ALU.mult, op1=ALU.add)
                if has_gamma:
                    nc.vector.tensor_mul(out=yn[:], in0=yn[:], in1=gamma_bc[:])
                if has_beta:
                    nc.vector.tensor_add(out=ob[:], in0=yn[:], in1=beta_bc[:])
                else:
                    nc.vector.tensor_copy(out=ob[:], in_=yn[:])
            else:
                nc.vector.tensor_scalar(out=ob[:], in0=y_b[:],
                                        scalar1=rstd[:, 1, bi:bi + 1],
                                        scalar2=nbias[:, bi:bi + 1],
                                        op0=ALU.mult, op1=ALU.add)
            nc.sync.dma_start(out=out_d[b * P:(b + 1) * P, :], in_=ob[:])

    # interleaved emission: q chunks, g2 (gated on the collectives), and
    # sweep-2 groups pipelined one chunk behind their matmuls
    _emit_qchunk(0)
    _emit_qchunk(1)
    _emit_g2()
    _emit_s2_mm(0)
    _emit_qchunk(2)
    _emit_s2_stats(0)
    _emit_s2_mm(1)
    _emit_qchunk(3)
    _emit_s2_stats(1)
    _emit_s2_mm(2)
    _emit_s2_stats(2)
    _emit_s2_mm(3)
    _emit_s2_stats(3)


_PROGRAM_CACHE = {}


def _build_program(flags):
    if flags in _PROGRAM_CACHE:
        return _PROGRAM_CACHE[flags]
    nc = bass.Bass("TRN2", target_bir_lowering=False, debug=False,
                   num_devices=NCORES)
    x_d = nc.dram_tensor("x_shard", [R, DM], BF16, kind="ExternalInput").ap()
    xt_d = nc.dram_tensor("xt8", [NCHUNK, P, NK2, 2, CHUNK], F8,
                          kind="ExternalInput").ap()
    wqt_d = nc.dram_tensor("wq8", [P, NK2, 2, DM], F8, kind="ExternalInput").ap()
    wkt_d = nc.dram_tensor("wk8", [P, NK2, 2, DM], F8, kind="ExternalInput").ap()
    wvt_d = nc.dram_tensor("wv8", [P, NK2, 2, DM], F8, kind="ExternalInput").ap()
    bq_d = nc.dram_tensor("bq", [DM], F32, kind="ExternalInput").ap()
    bk_d = nc.dram_tensor("bk", [DM], F32, kind="ExternalInput").ap()
    bv_d = nc.dram_tensor("bv", [DM], F32, kind="ExternalInput").ap()
    gamma_d = nc.dram_tensor("gamma", [DM], F32, kind="ExternalInput").ap()
    beta_d = nc.dram_tensor("beta", [DM], F32, kind="ExternalInput").ap()
    denmask_d = nc.dram_tensor("denmask", [P, NKT, H], F8,
                               kind="ExternalInput").ap()
    bcmask_d = nc.dram_tensor("bcmask", [16, NKT, P], BF16,
                              kind="ExternalInput").ap()
    ident_d = nc.dram_tensor("ident", [P, P], BF16, kind="ExternalInput").ap()
    out_d = nc.dram_tensor("out_shard", [R, DM], BF16, kind="ExternalOutput").ap()
    io = (x_d, xt_d, wqt_d, wkt_d, wvt_d, bq_d, bk_d, bv_d, gamma_d, beta_d,
          denmask_d, bcmask_d, ident_d, out_d)
    with tile.TileContext(nc) as tc:
        with ExitStack() as ctx:
            _body(ctx, tc, io, flags)
    _fix_multiwaits(nc)
    _PROGRAM_CACHE[flags] = nc
    return nc


def _prep_inputs(x, Wq, bq, Wk, bk, Wv, bv, gamma, beta):
    """Host-side: shard x, build fp8 layouts. Returns (in_maps, flags)."""
    import ml_dtypes
    f8 = ml_dtypes.float8_e4m3
    bf16 = ml_dtypes.bfloat16
    x = np.ascontiguousarray(np.asarray(x, dtype=np.float32))
    flags = (bool(np.any(bq)), bool(np.any(bk)), bool(np.any(bv)),
             bool(np.any(np.asarray(gamma) != 1.0)), bool(np.any(beta)))

    def _w8(W):
        # [P, NK2, 2, DM]: [p,t2,i,n] = SW * W[n, (2t2+i)*128+p]
        Wt = (np.asarray(W, dtype=np.float32).T * SW).astype(f8)  # [in, out]
        return np.ascontiguousarray(
            Wt.reshape(NK2, 2, P, DM).transpose(2, 0, 1, 3))

    # masks: predicate h == 2m + ch//64
    ch = np.arange(P)
    mm_, hh = np.meshgrid(np.arange(NKT), np.arange(H), indexing="ij")
    pred = (hh[None, :, :] == 2 * mm_[None, :, :] + (ch[:, None, None] // 64))
    denmask = pred.astype(np.float32).astype(f8)               # [P, NKT, H]
    bcmask = np.ascontiguousarray(
        pred.transpose(2, 1, 0).astype(np.float32).astype(bf16))  # [H,NKT,P]

    common = {
        "wq8": _w8(Wq), "wk8": _w8(Wk), "wv8": _w8(Wv),
        "denmask": np.ascontiguousarray(denmask),
        "bcmask": bcmask,
        "ident": np.eye(P, dtype=np.float32).astype(bf16),
        "bq": np.ascontiguousarray(bq, dtype=np.float32),
        "bk": np.ascontiguousarray(bk, dtype=np.float32),
        "bv": np.ascontiguousarray(bv, dtype=np.float32),
        "gamma": np.ascontiguousarray(gamma, dtype=np.float32),
        "beta": np.ascontiguousarray(beta, dtype=np.float32),
    }
    in_maps = []
    for c in range(NCORES):
        b, half = divmod(c, 2)
        shard = np.ascontiguousarray(x[b, half * R:(half + 1) * R, :])
        # xt8 [NCHUNK, P, NK2, 2, CHUNK]: [c,p,t2,i,r] = x[c*512+r, (2t2+i)*128+p]
        x8 = shard.astype(f8).reshape(NCHUNK, CHUNK, NK2, 2, P)
        x8 = np.ascontiguousarray(x8.transpose(0, 4, 2, 3, 1))
        in_maps.append({"x_shard": shard.astype(bf16), "xt8": x8, **common})
    return in_maps, flags


def kernel(x, mask, pad_mask, Wq, bq, Wk, bk, Wv, bv, gamma, beta):
    in_maps, flags = _prep_inputs(x, Wq, bq, Wk, bk, Wv, bv, gamma, beta)
    nc = _build_program(flags)
    res = run_bass_kernel_spmd(nc, in_maps, list(range(NCORES)))
    out = np.empty((B, S, DM), dtype=np.float32)
    for c in range(NCORES):
        b, half = divmod(c, 2)
        out[b, half * R:(half + 1) * R, :] = np.asarray(
            res.results[c]["out_shard"]).astype(np.float32)
    return out


if __name__ == "__main__":
    rng = np.random.default_rng(0)
    s = 1.0 / np.sqrt(DM)
    demo = {
        "x": rng.standard_normal((B, S, DM), dtype=np.float32),
        "mask": np.zeros((S, S), bool),
        "pad_mask": np.zeros((B, S), bool),
        "Wq": rng.uniform(-s, s, (DM, DM)).astype(np.float32),
        "bq": np.zeros(DM, np.float32),
        "Wk": rng.uniform(-s, s, (DM, DM)).astype(np.float32),
        "bk": np.zeros(DM, np.float32),
        "Wv": rng.uniform(-s, s, (DM, DM)).astype(np.float32),
        "bv": np.zeros(DM, np.float32),
        "gamma": np.ones(DM, np.float32),
        "beta": np.zeros(DM, np.float32),
    }
    out = kernel(**demo)
    print("out", out.shape, out.dtype, float(np.abs(out).max()))
